# revision 6
# baseline (speedup 1.0000x reference)
"""Multi-head self-attention on 8 Trainium2 NeuronCores.

Problem: B=2, S=2048, D=1024, H=16 heads (DK=64), fp32.

Sharding (8 cores): core c handles batch b = c//4 and head group g = c%4
(4 heads = 256 of the 1024 projection dims).  QKV are column-parallel,
Wo is row-parallel; the 4 partial outputs per batch are summed on the
host (cheap numpy add) together with a folded constant bias vector.

Device kernel (per core, identical SPMD program):
  - inputs are pre-transposed on host so no on-device transposes are needed:
      xT [1024, 2048], wqT/wkT/wvT [1024, 256], woT [256, 1024]
  - QK^T is computed in "scoresT" layout [kk, q] (both operands d-major),
    exp(scale*s + mask_bias) fused on ScalarE (PSUM -> SBUF),
  - P^T @ V' with a ones-column appended to V gives context^T and the
    softmax denominators in one PSUM accumulation,
  - per-head normalize via reciprocal + DMA partition-broadcast,
  - Wo projection of the normalized context^T -> partial out [2048, 1024].

All matmuls use float32r (fp32 data, single-pass PE) for 4x fp32 speed.

Math notes (exactness):
  - K bias cancels in softmax (adds a per-query constant to scores).
  - V bias commutes: softmax(S) @ (V + 1 b_v^T) = softmax(S) @ V + b_v^T,
    so it is added on the host as Wo_w @ Wv_b (+ Wo_b) once per batch.
"""

import sys

for _p in ("/root/.axon_site", "/root/.axon_site/_ro/trn_rl_repo",
           "/root/.axon_site/_ro/pypackages", "/opt/trn_rl_repo"):
    if _p not in sys.path:
        sys.path.append(_p)

import numpy as np

import concourse.bass as bass
import concourse.tile as tile
from concourse import bacc, mybir
from concourse.bass_utils import run_bass_kernel_spmd

B, S, D, H = 2, 2048, 1024, 16
DK = D // H          # 64 head dim
NCORES = 8
HL = H // 4          # 4 heads per core
CL = HL * DK         # 256 local context dims per core
P = 128
EC = D // P          # 8 contraction chunks
F32 = mybir.dt.float32
F32R = mybir.dt.float32r
AF = mybir.ActivationFunctionType

LAST_RESULT = None   # BassKernelResults of the most recent run (for test.py)


def build_program():
    nc = bacc.Bacc("TRN2", target_bir_lowering=False, debug=False,
                   num_devices=NCORES)
    xT = nc.dram_tensor("xT", [D, S], F32R, kind="ExternalInput")
    wqT = nc.dram_tensor("wqT", [D, CL], F32R, kind="ExternalInput")
    wkT = nc.dram_tensor("wkT", [D, CL], F32R, kind="ExternalInput")
    wvT = nc.dram_tensor("wvT", [D, CL], F32R, kind="ExternalInput")
    bq2 = nc.dram_tensor("bq2", [P, CL // P], F32, kind="ExternalInput")
    mb = nc.dram_tensor("mb", [P, S // P], F32, kind="ExternalInput")
    woT = nc.dram_tensor("woT", [CL, D], F32R, kind="ExternalInput")
    pout = nc.dram_tensor("pout", [S, D], F32, kind="ExternalOutput")

    KT_TILES = S // P       # 16 key tiles
    QC = 4                  # query chunks of 512
    QW = S // QC            # 512

    with tile.TileContext(nc) as tc:
        with (
            tc.tile_pool(name="consts", bufs=1) as consts,
            tc.tile_pool(name="psum", bufs=1, space="PSUM") as psum,
        ):
            # persistent SBUF tensors
            qt_sb = consts.tile([P, 2, S], F32R)          # Q^T  [256, 2048]
            kt_sb = consts.tile([P, 2, S], F32R)          # K^T  [256, 2048]
            v_sb = consts.tile([P, KT_TILES, HL, DK + 1], F32R)  # V + ones col
            ctxn = consts.tile([P, 2, S], F32R)           # normalized ctx^T
            bq_sb = consts.tile([P, 2], F32)
            mb_sb = consts.tile([P, KT_TILES], F32)
            wo_sb = consts.tile([P, 2, D], F32R)
            wq_sb = consts.tile([P, EC, CL], F32R)
            wk_sb = consts.tile([P, EC, CL], F32R)
            wv_sb = consts.tile([P, EC, CL], F32R)

            nc.sync.dma_start(out=wq_sb, in_=wqT.rearrange("(j p) c -> p j c", p=P))
            nc.sync.dma_start(out=wk_sb, in_=wkT.rearrange("(j p) c -> p j c", p=P))
            nc.sync.dma_start(out=wv_sb, in_=wvT.rearrange("(j p) c -> p j c", p=P))
            nc.sync.dma_start(out=bq_sb, in_=bq2[:, :])
            nc.sync.dma_start(out=mb_sb, in_=mb[:, :])
            nc.sync.dma_start(out=wo_sb, in_=woT.rearrange("(j p) c -> p j c", p=P))
            ones_sb = consts.tile([P, KT_TILES * HL], F32)
            nc.vector.memset(ones_sb, 1.0)
            nc.vector.tensor_copy(
                out=v_sb[:, :, :, DK],
                in_=ones_sb.rearrange("p (t h) -> p t h", h=HL))

            ptags = ["sc", "ctx"]  # the two 4-bank PSUM slots, reused all phases
            ti = 0

            with tc.tile_pool(name="xtp", bufs=1) as xtp:
                xt_sb = xtp.tile([P, EC, S], F32R)
                nc.sync.dma_start(out=xt_sb, in_=xT.rearrange("(j p) q -> p j q", p=P))

                # ---- Q/K projections (transposed outputs, d-major) ----
                for w_sb, o_sb, add_bias in ((wq_sb, qt_sb, True),
                                             (wk_sb, kt_sb, False)):
                    for j in range(2):
                        for qc in range(QC):
                            ps = psum.tile([P, QW], F32, tag=ptags[ti % 2],
                                           name=f"pj{ti}")
                            ti += 1
                            for e in range(EC):
                                nc.tensor.matmul(
                                    ps,
                                    lhsT=w_sb[:, e, j * P:(j + 1) * P],
                                    rhs=xt_sb[:, e, qc * QW:(qc + 1) * QW],
                                    start=(e == 0), stop=(e == EC - 1))
                            dst = o_sb[:, j, qc * QW:(qc + 1) * QW]
                            if add_bias:
                                nc.vector.tensor_scalar_add(
                                    out=dst, in0=ps, scalar1=bq_sb[:, j:j + 1])
                            else:
                                nc.vector.tensor_copy(out=dst, in_=ps)

                # ---- V projection (natural layout, kk-major) ----
                for kt in range(KT_TILES):
                    ps = psum.tile([P, CL], F32, tag=ptags[ti % 2], name=f"pv{ti}")
                    ti += 1
                    for e in range(EC):
                        nc.tensor.matmul(
                            ps,
                            lhsT=xt_sb[:, e, kt * P:(kt + 1) * P],
                            rhs=wv_sb[:, e, :],
                            start=(e == 0), stop=(e == EC - 1))
                    nc.vector.tensor_copy(
                        out=v_sb[:, kt, :, 0:DK],
                        in_=ps.rearrange("p (h d) -> p h d", h=HL))

            # ---- attention (per local head) ----
            with (
                tc.tile_pool(name="attp", bufs=1) as attp,
                tc.tile_pool(name="dramp", bufs=2, space="DRAM") as dramp,
            ):
                for h in range(HL):
                    hb, hr = h // 2, (h % 2) * DK
                    QT = qt_sb[hr:hr + DK, hb, :]
                    KT = kt_sb[hr:hr + DK, hb, :]
                    ctx_ps = psum.tile([P, S], F32, tag="ctx", name=f"ctx{h}")
                    for kt in range(KT_TILES):
                        sc_ps = psum.tile([P, S], F32, tag="sc", name=f"sc{h}_{kt}")
                        for qc in range(QC):
                            nc.tensor.matmul(
                                sc_ps[:, qc * QW:(qc + 1) * QW],
                                lhsT=KT[:, kt * P:(kt + 1) * P],
                                rhs=QT[:, qc * QW:(qc + 1) * QW],
                                start=True, stop=True)
                        pt = attp.tile([P, S], F32R, tag="pt", bufs=3,
                                       name=f"pt{h}_{kt}")
                        # P^T = exp(s/sqrt(dk) + mask_bias), fused on ScalarE
                        nc.scalar.activation(out=pt, in_=sc_ps, func=AF.Exp,
                                             bias=mb_sb[:, kt:kt + 1],
                                             scale=1.0 / float(np.sqrt(DK)))
                        for qc in range(QC):
                            nc.tensor.matmul(
                                ctx_ps[0:DK + 1, qc * QW:(qc + 1) * QW],
                                lhsT=v_sb[:, kt, h, :],
                                rhs=pt[:, qc * QW:(qc + 1) * QW],
                                start=(kt == 0), stop=(kt == KT_TILES - 1))
                    # normalize: ctx[d, q] * (1 / denom[q]) with denom = row DK
                    rd = attp.tile([1, S], F32, tag="rd", bufs=2, name=f"rd{h}")
                    nc.vector.reciprocal(out=rd, in_=ctx_ps[DK:DK + 1, :])
                    rdd = dramp.tile([1, S], F32, tag="rdd", name=f"rdd{h}")
                    nc.sync.dma_start(out=rdd, in_=rd)
                    rb = attp.tile([DK, S], F32, tag="rb", bufs=2, name=f"rb{h}")
                    nc.sync.dma_start(out=rb, in_=rdd.to_broadcast([DK, S]))
                    nc.vector.tensor_mul(out=ctxn[hr:hr + DK, hb, :],
                                         in0=ctx_ps[0:DK, :], in1=rb)

                # ---- output projection ----
                for t in range(S // P):
                    po = attp.tile([P, D], F32, tag="po", bufs=3, name=f"po{t}")
                    for dc in range(2):
                        ps = psum.tile([P, QW], F32, tag=ptags[dc], name=f"po{t}_{dc}")
                        for cb in range(2):
                            nc.tensor.matmul(
                                ps,
                                lhsT=ctxn[:, cb, t * P:(t + 1) * P],
                                rhs=wo_sb[:, cb, dc * QW:(dc + 1) * QW],
                                start=(cb == 0), stop=(cb == 1))
                        nc.vector.tensor_copy(out=po[:, dc * QW:(dc + 1) * QW], in_=ps)
                    nc.sync.dma_start(out=pout[t * P:(t + 1) * P, :], in_=po)

    nc.compile()
    return nc


_PROGRAM = None


def _get_program():
    global _PROGRAM
    if _PROGRAM is None:
        _PROGRAM = build_program()
    return _PROGRAM


def kernel(x, mask, Wq_w, Wq_b, Wk_w, Wk_b, Wv_w, Wv_b, Wo_w, Wo_b,
           **run_kwargs):
    global LAST_RESULT
    x = np.asarray(x, np.float32)
    mask = np.asarray(mask)
    Wq_w = np.asarray(Wq_w, np.float32)
    Wk_w = np.asarray(Wk_w, np.float32)
    Wv_w = np.asarray(Wv_w, np.float32)
    Wo_w = np.asarray(Wo_w, np.float32)

    nc = _get_program()

    xTs = [np.ascontiguousarray(x[b].T) for b in range(B)]
    mbs = []
    for b in range(B):
        mrow = np.asarray(mask[b, 0, 0, :])
        bias = np.where(mrow == 0, np.float32(-50.0), np.float32(0.0))
        mbs.append(np.ascontiguousarray(bias.reshape(S // P, P).T.astype(np.float32)))

    in_maps = []
    for c in range(NCORES):
        b, g = c // 4, c % 4
        sl = slice(g * CL, (g + 1) * CL)
        in_maps.append({
            "xT": xTs[b],
            "wqT": np.ascontiguousarray(Wq_w[sl, :].T),
            "wkT": np.ascontiguousarray(Wk_w[sl, :].T),
            "wvT": np.ascontiguousarray(Wv_w[sl, :].T),
            "bq2": np.ascontiguousarray(
                np.asarray(Wq_b, np.float32)[sl].reshape(CL // P, P).T),
            "mb": mbs[b],
            "woT": np.ascontiguousarray(Wo_w[:, sl].T),
        })

    res = run_bass_kernel_spmd(nc, in_maps, core_ids=list(range(NCORES)),
                               **run_kwargs)
    LAST_RESULT = res

    # host-side unshard: sum the 4 row-parallel partials per batch and add
    # the folded constant bias (Wo @ Wv_b + Wo_b).
    obias = (Wo_w @ np.asarray(Wv_b, np.float32)
             + np.asarray(Wo_b, np.float32)).astype(np.float32)
    out = np.empty((B, S, D), np.float32)
    for b in range(B):
        acc = res.results[4 * b]["pout"].astype(np.float32)
        for g in range(1, 4):
            acc = acc + res.results[4 * b + g]["pout"]
        out[b] = acc + obias
    return out


# revision 7
# speedup vs baseline: 1.1142x; 1.1142x over previous
"""Multi-head self-attention on 8 Trainium2 NeuronCores.

Problem: B=2, S=2048, D=1024, H=16 heads (DK=64), fp32.

Sharding (8 cores): core c handles batch b = c//4 and head group g = c%4
(4 heads = 256 of the 1024 projection dims).  QKV are column-parallel,
Wo is row-parallel; the 4 partial outputs per batch are summed on the
host (cheap numpy add) together with a folded constant bias vector.

Device kernel (per core, identical SPMD program):
  - inputs are pre-transposed AND pre-cast to bf16 on host so no
    on-device transposes are needed:
      xT [1024, 2048], wqT/wkT/wvT [1024, 256], woT [256, 1024]
  - QK^T is computed in "scoresT" layout [kk, q] (both operands d-major),
    exp(scale*s + mask_bias) fused on ScalarE (PSUM -> SBUF, bf16 out),
  - P^T @ V' with a ones-column appended to V gives context^T and the
    softmax denominators in one PSUM accumulation,
  - context is evicted unnormalized (frees PSUM for the next head);
    the denominator row is reshaped to [128, 16] via a DRAM bounce so
    the iterative-divide reciprocal runs on 128 lanes, then partition-
    broadcast back and applied with one tensor-tensor multiply,
  - Wo projection of the normalized context^T -> partial out [2048, 1024].

bf16 matmul operands: full-rate PE (1 cycle/row) with FWL weight loads
that overlap in-flight matmuls (the fp32/f32r paths inline a serial
4-byte weight load per matmul, which measured ~194us of extra PE time).

Math notes (exactness):
  - K bias cancels in softmax (adds a per-query constant to scores).
  - V bias commutes: softmax(S) @ (V + 1 b_v^T) = softmax(S) @ V + b_v^T,
    so it is added on the host as Wo_w @ Wv_b (+ Wo_b) once per batch.
"""

import sys

for _p in ("/root/.axon_site", "/root/.axon_site/_ro/trn_rl_repo",
           "/root/.axon_site/_ro/pypackages", "/opt/trn_rl_repo"):
    if _p not in sys.path:
        sys.path.append(_p)

import ml_dtypes
import numpy as np

import concourse.bass as bass
import concourse.tile as tile
from concourse import bacc, mybir
from concourse.bass_utils import run_bass_kernel_spmd

B, S, D, H = 2, 2048, 1024, 16
DK = D // H          # 64 head dim
NCORES = 8
HL = H // 4          # 4 heads per core
CL = HL * DK         # 256 local context dims per core
P = 128
EC = D // P          # 8 contraction chunks
F32 = mybir.dt.float32
BF16 = mybir.dt.bfloat16
AF = mybir.ActivationFunctionType
BF = ml_dtypes.bfloat16

LAST_RESULT = None   # BassKernelResults of the most recent run (for test.py)


def build_program():
    nc = bacc.Bacc("TRN2", target_bir_lowering=False, debug=False,
                   num_devices=NCORES)
    xT = nc.dram_tensor("xT", [D, S], BF16, kind="ExternalInput")
    wqT = nc.dram_tensor("wqT", [D, CL], BF16, kind="ExternalInput")
    wkT = nc.dram_tensor("wkT", [D, CL], BF16, kind="ExternalInput")
    wvT = nc.dram_tensor("wvT", [D, CL], BF16, kind="ExternalInput")
    bq2 = nc.dram_tensor("bq2", [P, CL // P], F32, kind="ExternalInput")
    mb = nc.dram_tensor("mb", [P, S // P], F32, kind="ExternalInput")
    woT = nc.dram_tensor("woT", [CL, D], BF16, kind="ExternalInput")
    pout = nc.dram_tensor("pout", [S, D], F32, kind="ExternalOutput")

    KT_TILES = S // P       # 16 key tiles
    QC = 4                  # query chunks of 512
    QW = S // QC            # 512

    with tile.TileContext(nc) as tc:
        with (
            tc.tile_pool(name="consts", bufs=1) as consts,
            tc.tile_pool(name="psum", bufs=1, space="PSUM") as psum,
        ):
            # persistent SBUF tensors
            qt_sb = consts.tile([P, 2, S], BF16)          # Q^T  [256, 2048]
            kt_sb = consts.tile([P, 2, S], BF16)          # K^T  [256, 2048]
            v_sb = consts.tile([P, KT_TILES, HL, DK + 1], BF16)  # V + ones col
            ctxn = consts.tile([P, 2, S], BF16)           # normalized ctx^T
            bq_sb = consts.tile([P, 2], F32)
            mb_sb = consts.tile([P, KT_TILES], F32)
            wo_sb = consts.tile([P, 2, D], BF16)
            wq_sb = consts.tile([P, EC, CL], BF16)
            wk_sb = consts.tile([P, EC, CL], BF16)
            wv_sb = consts.tile([P, EC, CL], BF16)

            nc.sync.dma_start(out=wq_sb, in_=wqT.rearrange("(j p) c -> p j c", p=P))
            nc.sync.dma_start(out=wk_sb, in_=wkT.rearrange("(j p) c -> p j c", p=P))
            nc.sync.dma_start(out=wv_sb, in_=wvT.rearrange("(j p) c -> p j c", p=P))
            nc.sync.dma_start(out=bq_sb, in_=bq2[:, :])
            nc.sync.dma_start(out=mb_sb, in_=mb[:, :])
            nc.sync.dma_start(out=wo_sb, in_=woT.rearrange("(j p) c -> p j c", p=P))
            nc.vector.memset(v_sb[:, :, :, DK:DK + 1], 1.0)

            ptags = ["sc", "ctx"]  # the two 4-bank PSUM slots, reused all phases
            ti = 0

            with tc.tile_pool(name="xtp", bufs=1) as xtp:
                xt_sb = xtp.tile([P, EC, S], BF16)
                nc.sync.dma_start(out=xt_sb, in_=xT.rearrange("(j p) q -> p j q", p=P))

                # ---- Q/K projections (transposed outputs, d-major) ----
                for w_sb, o_sb, add_bias in ((wq_sb, qt_sb, True),
                                             (wk_sb, kt_sb, False)):
                    for j in range(2):
                        for qc in range(QC):
                            ps = psum.tile([P, QW], F32, tag=ptags[ti % 2],
                                           name=f"pj{ti}")
                            ti += 1
                            for e in range(EC):
                                nc.tensor.matmul(
                                    ps,
                                    lhsT=w_sb[:, e, j * P:(j + 1) * P],
                                    rhs=xt_sb[:, e, qc * QW:(qc + 1) * QW],
                                    start=(e == 0), stop=(e == EC - 1))
                            dst = o_sb[:, j, qc * QW:(qc + 1) * QW]
                            if add_bias:
                                nc.vector.tensor_scalar_add(
                                    out=dst, in0=ps, scalar1=bq_sb[:, j:j + 1])
                            else:
                                nc.vector.tensor_copy(out=dst, in_=ps)

                # ---- V projection (natural layout, kk-major) ----
                for kt in range(KT_TILES):
                    ps = psum.tile([P, CL], F32, tag=ptags[ti % 2], name=f"pv{ti}")
                    ti += 1
                    for e in range(EC):
                        nc.tensor.matmul(
                            ps,
                            lhsT=xt_sb[:, e, kt * P:(kt + 1) * P],
                            rhs=wv_sb[:, e, :],
                            start=(e == 0), stop=(e == EC - 1))
                    nc.vector.tensor_copy(
                        out=v_sb[:, kt, :, 0:DK],
                        in_=ps.rearrange("p (h d) -> p h d", h=HL))

            # ---- attention (per local head) ----
            with (
                tc.tile_pool(name="attp", bufs=1) as attp,
                tc.tile_pool(name="dramp", bufs=2, space="DRAM") as dramp,
            ):
                for h in range(HL):
                    hb, hr = h // 2, (h % 2) * DK
                    QT = qt_sb[hr:hr + DK, hb, :]
                    KT = kt_sb[hr:hr + DK, hb, :]
                    ctx_ps = psum.tile([P, S], F32, tag="ctx", name=f"ctx{h}")
                    for kt in range(KT_TILES):
                        sc_ps = psum.tile([P, S], F32, tag="sc", name=f"sc{h}_{kt}")
                        for qc in range(QC):
                            nc.tensor.matmul(
                                sc_ps[:, qc * QW:(qc + 1) * QW],
                                lhsT=KT[:, kt * P:(kt + 1) * P],
                                rhs=QT[:, qc * QW:(qc + 1) * QW],
                                start=True, stop=True)
                        pt = attp.tile([P, S], BF16, tag="pt", bufs=3,
                                       name=f"pt{h}_{kt}")
                        # P^T = exp(s/sqrt(dk) + mask_bias), fused on ScalarE
                        nc.scalar.activation(out=pt, in_=sc_ps, func=AF.Exp,
                                             bias=mb_sb[:, kt:kt + 1],
                                             scale=1.0 / float(np.sqrt(DK)))
                        for qc in range(QC):
                            nc.tensor.matmul(
                                ctx_ps[0:DK + 1, qc * QW:(qc + 1) * QW],
                                lhsT=v_sb[:, kt, h, :],
                                rhs=pt[:, qc * QW:(qc + 1) * QW],
                                start=(kt == 0), stop=(kt == KT_TILES - 1))

                    # evict context unnormalized (frees the PSUM slot fast)
                    ctxu = attp.tile([DK, S], F32, tag="ctxu", bufs=2,
                                     name=f"ctxu{h}")
                    nc.vector.tensor_copy(out=ctxu, in_=ctx_ps[0:DK, :])
                    den = attp.tile([1, S], F32, tag="den", bufs=2, name=f"den{h}")
                    nc.scalar.copy(out=den, in_=ctx_ps[DK:DK + 1, :])

                    # reciprocal on a [128, 16] reshape (iterative divide is
                    # ~8 cyc/elem per lane; [1, 2048] would cost ~13 us)
                    dd = dramp.tile([1, S], F32, tag="dd", name=f"dd{h}")
                    nc.sync.dma_start(out=dd, in_=den)
                    den2 = attp.tile([P, S // P], F32, tag="den2", bufs=2,
                                     name=f"den2{h}")
                    nc.sync.dma_start(
                        out=den2, in_=dd.rearrange("o (p f) -> (o p) f", p=P))
                    den2r = attp.tile([P, S // P], F32, tag="den2r", bufs=2,
                                      name=f"den2r{h}")
                    nc.vector.reciprocal(out=den2r, in_=den2)
                    dr = dramp.tile([1, S], F32, tag="dr", name=f"dr{h}")
                    nc.sync.dma_start(
                        out=dr.rearrange("o (p f) -> (o p) f", p=P), in_=den2r)
                    rb = attp.tile([DK, S], F32, tag="rb", bufs=2, name=f"rb{h}")
                    nc.sync.dma_start(out=rb, in_=dr.to_broadcast([DK, S]))
                    nc.vector.tensor_mul(out=ctxn[hr:hr + DK, hb, :],
                                         in0=ctxu, in1=rb)

                # ---- output projection ----
                for t in range(S // P):
                    po = attp.tile([P, D], F32, tag="po", bufs=3, name=f"po{t}")
                    for dc in range(2):
                        ps = psum.tile([P, QW], F32, tag=ptags[dc], name=f"pw{t}_{dc}")
                        for cb in range(2):
                            nc.tensor.matmul(
                                ps,
                                lhsT=ctxn[:, cb, t * P:(t + 1) * P],
                                rhs=wo_sb[:, cb, dc * QW:(dc + 1) * QW],
                                start=(cb == 0), stop=(cb == 1))
                        nc.vector.tensor_copy(out=po[:, dc * QW:(dc + 1) * QW], in_=ps)
                    nc.sync.dma_start(out=pout[t * P:(t + 1) * P, :], in_=po)

    nc.compile()
    return nc


_PROGRAM = None


def _get_program():
    global _PROGRAM
    if _PROGRAM is None:
        _PROGRAM = build_program()
    return _PROGRAM


def _bf(a):
    return np.ascontiguousarray(np.asarray(a, np.float32)).astype(BF)


def kernel(x, mask, Wq_w, Wq_b, Wk_w, Wk_b, Wv_w, Wv_b, Wo_w, Wo_b,
           **run_kwargs):
    global LAST_RESULT
    x = np.asarray(x, np.float32)
    mask = np.asarray(mask)
    Wq_w = np.asarray(Wq_w, np.float32)
    Wk_w = np.asarray(Wk_w, np.float32)
    Wv_w = np.asarray(Wv_w, np.float32)
    Wo_w = np.asarray(Wo_w, np.float32)

    nc = _get_program()

    xTs = [_bf(x[b].T) for b in range(B)]
    mbs = []
    for b in range(B):
        mrow = np.asarray(mask[b, 0, 0, :])
        bias = np.where(mrow == 0, np.float32(-50.0), np.float32(0.0))
        mbs.append(np.ascontiguousarray(bias.reshape(S // P, P).T.astype(np.float32)))

    in_maps = []
    for c in range(NCORES):
        b, g = c // 4, c % 4
        sl = slice(g * CL, (g + 1) * CL)
        in_maps.append({
            "xT": xTs[b],
            "wqT": _bf(Wq_w[sl, :].T),
            "wkT": _bf(Wk_w[sl, :].T),
            "wvT": _bf(Wv_w[sl, :].T),
            "bq2": np.ascontiguousarray(
                np.asarray(Wq_b, np.float32)[sl].reshape(CL // P, P).T),
            "mb": mbs[b],
            "woT": _bf(Wo_w[:, sl].T),
        })

    res = run_bass_kernel_spmd(nc, in_maps, core_ids=list(range(NCORES)),
                               **run_kwargs)
    LAST_RESULT = res

    # host-side unshard: sum the 4 row-parallel partials per batch and add
    # the folded constant bias (Wo @ Wv_b + Wo_b).
    obias = (Wo_w @ np.asarray(Wv_b, np.float32)
             + np.asarray(Wo_b, np.float32)).astype(np.float32)
    out = np.empty((B, S, D), np.float32)
    for b in range(B):
        acc = res.results[4 * b]["pout"].astype(np.float32)
        for g in range(1, 4):
            acc = acc + res.results[4 * b + g]["pout"]
        out[b] = acc + obias
    return out


# revision 9
# speedup vs baseline: 1.7433x; 1.5646x over previous
"""Multi-head self-attention on 8 Trainium2 NeuronCores.

Problem: B=2, S=2048, D=1024, H=16 heads (DK=64), fp32.

Sharding (8 cores): core c handles batch b = c//4 and head group g = c%4
(4 heads = 256 of the 1024 projection dims).  QKV are column-parallel,
Wo is row-parallel; the 4 partial outputs per batch are summed on the
host (cheap numpy add) together with a folded constant bias vector.

Device kernel (per core, identical SPMD program), bf16 matmul operands:
  - inputs are pre-transposed and pre-cast to bf16 on host (no on-device
    transposes): xT [1024,2048], wqT/wkT/wvT [1024,256], woT [256,1024].
  - V is projected for all 4 local heads up front; Q^T/K^T are projected
    PER HEAD, software-pipelined as TensorE filler inside the previous
    head's attention loop.  This keeps the PE continuously busy while
    ScalarE runs the exps, so the HAM clock gate stays at 2.4 GHz
    (an ACT-bound attention loop lets the PE micro-idle, HAM rethrottles
    to 1.2 GHz, and every matmul doubles in cost — measured 485us).
  - scores^T layout [kk, q] per (head, q-half): matmul -> PSUM[128,1024],
    exp(s/8 + mask_bias) fused on ScalarE -> bf16 P^T tiles,
    P^T @ V' (ones-column appended to V) accumulates context^T and the
    softmax denominators in one PSUM tile.
  - context is evicted unnormalized; the denominator row is reshaped to
    [128, 16] via a DRAM bounce so the iterative-divide reciprocal runs
    on 128 lanes (a [1, 2048] reciprocal costs ~13us), then broadcast
    back along partitions by DMA and applied with one tensor multiply.
  - Wo projection of normalized context^T -> partial out [2048, 1024].

PSUM budget (8 banks): score tiles [128,1024] x2 bufs (4) + context
accumulator [128,1024] (2) + projection tiles [128,512] x2 bufs (2).

Math notes (exactness):
  - K bias cancels in softmax (adds a per-query constant to scores).
  - V bias commutes: softmax(S) @ (V + 1 b_v^T) = softmax(S) @ V + b_v^T,
    so it is added on the host as Wo_w @ Wv_b (+ Wo_b) once per batch.
"""

import sys

for _p in ("/root/.axon_site", "/root/.axon_site/_ro/trn_rl_repo",
           "/root/.axon_site/_ro/pypackages", "/opt/trn_rl_repo"):
    if _p not in sys.path:
        sys.path.append(_p)

import ml_dtypes
import numpy as np

import concourse.bass as bass
import concourse.tile as tile
from concourse import bacc, mybir
from concourse.bass_utils import run_bass_kernel_spmd

B, S, D, H = 2, 2048, 1024, 16
DK = D // H          # 64 head dim
NCORES = 8
HL = H // 4          # 4 heads per core
CL = HL * DK         # 256 local context dims per core
P = 128
EC = D // P          # 8 contraction chunks
F32 = mybir.dt.float32
BF16 = mybir.dt.bfloat16
AF = mybir.ActivationFunctionType
BF = ml_dtypes.bfloat16

KT_TILES = S // P    # 16 key tiles
QW = 512             # matmul moving-dim chunk
SCW = 1024           # score-tile q width (one PSUM score tile)
NQH = S // SCW       # 2 q-halves per head

LAST_RESULT = None   # BassKernelResults of the most recent run (for test.py)


def build_program():
    nc = bacc.Bacc("TRN2", target_bir_lowering=False, debug=False,
                   num_devices=NCORES)
    xT = nc.dram_tensor("xT", [D, S], BF16, kind="ExternalInput")
    wqT = nc.dram_tensor("wqT", [D, CL], BF16, kind="ExternalInput")
    wkT = nc.dram_tensor("wkT", [D, CL], BF16, kind="ExternalInput")
    wvT = nc.dram_tensor("wvT", [D, CL], BF16, kind="ExternalInput")
    bq4 = nc.dram_tensor("bq4", [DK, HL], F32, kind="ExternalInput")
    mb = nc.dram_tensor("mb", [P, KT_TILES], F32, kind="ExternalInput")
    woT = nc.dram_tensor("woT", [CL, D], BF16, kind="ExternalInput")
    pout = nc.dram_tensor("pout", [S, D], F32, kind="ExternalOutput")

    with tile.TileContext(nc) as tc:
        with (
            tc.tile_pool(name="consts", bufs=1) as consts,
            tc.tile_pool(name="work", bufs=1) as work,
            tc.tile_pool(name="psum", bufs=1, space="PSUM") as psum,
            tc.tile_pool(name="dramp", bufs=2, space="DRAM") as dramp,
        ):
            # persistent SBUF tensors
            xt_sb = consts.tile([P, EC, S], BF16)
            wq_sb = consts.tile([P, EC, CL], BF16)
            wk_sb = consts.tile([P, EC, CL], BF16)
            wv_sb = consts.tile([P, EC, CL], BF16)
            v_sb = consts.tile([P, KT_TILES, HL, DK + 1], BF16)  # V + ones col
            ctxn = consts.tile([P, 2, S], BF16)                  # normalized ctx^T
            bq_sb = consts.tile([DK, HL], F32)
            mb_sb = consts.tile([P, KT_TILES], F32)
            wo_sb = consts.tile([P, 2, D], BF16)

            nc.sync.dma_start(out=xt_sb, in_=xT.rearrange("(j p) q -> p j q", p=P))
            nc.sync.dma_start(out=wq_sb, in_=wqT.rearrange("(j p) c -> p j c", p=P))
            nc.sync.dma_start(out=wk_sb, in_=wkT.rearrange("(j p) c -> p j c", p=P))
            nc.sync.dma_start(out=wv_sb, in_=wvT.rearrange("(j p) c -> p j c", p=P))
            nc.sync.dma_start(out=bq_sb, in_=bq4[:, :])
            nc.sync.dma_start(out=mb_sb, in_=mb[:, :])
            nc.sync.dma_start(out=wo_sb, in_=woT.rearrange("(j p) c -> p j c", p=P))
            nc.vector.memset(v_sb[:, :, :, DK:DK + 1], 1.0)

            # ---- V projection, all local heads up front ----
            for kt in range(KT_TILES):
                ps = psum.tile([P, QW], F32, tag="pj", bufs=2, name=f"pv{kt}")
                for e in range(EC):
                    nc.tensor.matmul(
                        ps[:, 0:CL],
                        lhsT=xt_sb[:, e, kt * P:(kt + 1) * P],
                        rhs=wv_sb[:, e, :],
                        start=(e == 0), stop=(e == EC - 1))
                nc.vector.tensor_copy(
                    out=v_sb[:, kt, :, 0:DK],
                    in_=ps[:, 0:CL].rearrange("p (h d) -> p h d", h=HL))

            # Per-head Q^T/K^T projection emitters.  Each returns a list of
            # closures (one matmul group each) so the caller can interleave
            # them as TensorE filler inside the previous head's attention.
            def qk_groups(h, qt_t, kt_t):
                groups = []
                for w_sb, o_t, is_q in ((wq_sb, qt_t, True), (wk_sb, kt_t, False)):
                    for qc in range(S // QW):
                        def g(w_sb=w_sb, o_t=o_t, is_q=is_q, qc=qc, h=h):
                            ps = psum.tile([P, QW], F32, tag="pj", bufs=2,
                                           name=f"pqk{h}_{int(is_q)}_{qc}")
                            for e in range(EC):
                                nc.tensor.matmul(
                                    ps[0:DK, :],
                                    lhsT=w_sb[:, e, h * DK:(h + 1) * DK],
                                    rhs=xt_sb[:, e, qc * QW:(qc + 1) * QW],
                                    start=(e == 0), stop=(e == EC - 1))
                            dst = o_t[:, qc * QW:(qc + 1) * QW]
                            if is_q:
                                nc.vector.tensor_scalar_add(
                                    out=dst, in0=ps[0:DK, :],
                                    scalar1=bq_sb[:, h:h + 1])
                            else:
                                nc.vector.tensor_copy(out=dst, in_=ps[0:DK, :])
                        groups.append(g)
                return groups

            def alloc_qk(h):
                qt_t = work.tile([DK, S], BF16, tag="qt", bufs=2, name=f"qt{h}")
                kt_t = work.tile([DK, S], BF16, tag="kt", bufs=2, name=f"kt{h}")
                return qt_t, kt_t

            # head 0's projections run up front
            cur_qk = alloc_qk(0)
            for g in qk_groups(0, *cur_qk):
                g()

            scale = 1.0 / float(np.sqrt(DK))
            for h in range(HL):
                qt_t, kt_t = cur_qk
                if h + 1 < HL:
                    nxt_qk = alloc_qk(h + 1)
                    filler = qk_groups(h + 1, *nxt_qk)
                else:
                    nxt_qk, filler = None, []
                fi = 0

                ctxu = work.tile([DK, S], F32, tag="ctxu", bufs=2, name=f"ctxu{h}")
                den = work.tile([1, S], F32, tag="den", bufs=2, name=f"den{h}")
                it = 0
                for qh in range(NQH):
                    q0 = qh * SCW
                    ctx_ps = psum.tile([P, SCW], F32, tag="ctx", bufs=1,
                                       name=f"ctx{h}_{qh}")
                    for kt in range(KT_TILES):
                        sc_ps = psum.tile([P, SCW], F32, tag="sc", bufs=2,
                                          name=f"sc{h}_{qh}_{kt}")
                        for c in range(SCW // QW):
                            nc.tensor.matmul(
                                sc_ps[:, c * QW:(c + 1) * QW],
                                lhsT=kt_t[:, kt * P:(kt + 1) * P],
                                rhs=qt_t[:, q0 + c * QW:q0 + (c + 1) * QW],
                                start=True, stop=True)
                        pt = work.tile([P, SCW], BF16, tag="pt", bufs=3,
                                       name=f"pt{h}_{qh}_{kt}")
                        nc.scalar.activation(out=pt, in_=sc_ps, func=AF.Exp,
                                             bias=mb_sb[:, kt:kt + 1],
                                             scale=scale)
                        for c in range(SCW // QW):
                            nc.tensor.matmul(
                                ctx_ps[0:DK + 1, c * QW:(c + 1) * QW],
                                lhsT=v_sb[:, kt, h, :],
                                rhs=pt[:, c * QW:(c + 1) * QW],
                                start=(kt == 0), stop=(kt == KT_TILES - 1))
                        # TensorE filler: next head's Q/K projection groups
                        if it % 4 == 1 and fi < len(filler):
                            filler[fi]()
                            fi += 1
                        it += 1
                    # evict unnormalized context + denominator row
                    nc.vector.tensor_copy(out=ctxu[:, q0:q0 + SCW],
                                          in_=ctx_ps[0:DK, :])
                    nc.scalar.copy(out=den[:, q0:q0 + SCW],
                                   in_=ctx_ps[DK:DK + 1, :])
                while fi < len(filler):
                    filler[fi]()
                    fi += 1
                cur_qk = nxt_qk

                # normalize: reciprocal on [128, 16] via DRAM bounce, then
                # partition-broadcast back and one multiply.
                dd = dramp.tile([1, S], F32, tag="dd", name=f"dd{h}")
                nc.sync.dma_start(out=dd, in_=den)
                den2 = work.tile([P, S // P], F32, tag="den2", bufs=2,
                                 name=f"den2{h}")
                nc.sync.dma_start(
                    out=den2, in_=dd.rearrange("o (p f) -> (o p) f", p=P))
                den2r = work.tile([P, S // P], F32, tag="den2r", bufs=2,
                                  name=f"den2r{h}")
                nc.vector.reciprocal(out=den2r, in_=den2)
                dr = dramp.tile([1, S], F32, tag="dr", name=f"dr{h}")
                nc.sync.dma_start(
                    out=dr.rearrange("o (p f) -> (o p) f", p=P), in_=den2r)
                rb = work.tile([DK, S], F32, tag="rb", bufs=2, name=f"rb{h}")
                nc.sync.dma_start(out=rb, in_=dr.to_broadcast([DK, S]))
                hb, hr = h // 2, (h % 2) * DK
                nc.vector.tensor_mul(out=ctxn[hr:hr + DK, hb, :],
                                     in0=ctxu, in1=rb)

            # ---- output projection ----
            for t in range(S // P):
                po = work.tile([P, D], F32, tag="po", bufs=3, name=f"po{t}")
                for dc in range(2):
                    ps = psum.tile([P, QW], F32, tag="pj", bufs=2,
                                   name=f"pw{t}_{dc}")
                    for cb in range(2):
                        nc.tensor.matmul(
                            ps,
                            lhsT=ctxn[:, cb, t * P:(t + 1) * P],
                            rhs=wo_sb[:, cb, dc * QW:(dc + 1) * QW],
                            start=(cb == 0), stop=(cb == 1))
                    nc.vector.tensor_copy(out=po[:, dc * QW:(dc + 1) * QW], in_=ps)
                nc.sync.dma_start(out=pout[t * P:(t + 1) * P, :], in_=po)

    nc.compile()
    return nc


_PROGRAM = None


def _get_program():
    global _PROGRAM
    if _PROGRAM is None:
        _PROGRAM = build_program()
    return _PROGRAM


def _bf(a):
    return np.ascontiguousarray(np.asarray(a, np.float32)).astype(BF)


def kernel(x, mask, Wq_w, Wq_b, Wk_w, Wk_b, Wv_w, Wv_b, Wo_w, Wo_b,
           **run_kwargs):
    global LAST_RESULT
    x = np.asarray(x, np.float32)
    mask = np.asarray(mask)
    Wq_w = np.asarray(Wq_w, np.float32)
    Wk_w = np.asarray(Wk_w, np.float32)
    Wv_w = np.asarray(Wv_w, np.float32)
    Wo_w = np.asarray(Wo_w, np.float32)

    nc = _get_program()

    xTs = [_bf(x[b].T) for b in range(B)]
    mbs = []
    for b in range(B):
        mrow = np.asarray(mask[b, 0, 0, :])
        bias = np.where(mrow == 0, np.float32(-50.0), np.float32(0.0))
        mbs.append(np.ascontiguousarray(bias.reshape(S // P, P).T.astype(np.float32)))

    in_maps = []
    for c in range(NCORES):
        b, g = c // 4, c % 4
        sl = slice(g * CL, (g + 1) * CL)
        in_maps.append({
            "xT": xTs[b],
            "wqT": _bf(Wq_w[sl, :].T),
            "wkT": _bf(Wk_w[sl, :].T),
            "wvT": _bf(Wv_w[sl, :].T),
            "bq4": np.ascontiguousarray(
                np.asarray(Wq_b, np.float32)[sl].reshape(HL, DK).T),
            "mb": mbs[b],
            "woT": _bf(Wo_w[:, sl].T),
        })

    res = run_bass_kernel_spmd(nc, in_maps, core_ids=list(range(NCORES)),
                               **run_kwargs)
    LAST_RESULT = res

    # host-side unshard: sum the 4 row-parallel partials per batch and add
    # the folded constant bias (Wo @ Wv_b + Wo_b).
    obias = (Wo_w @ np.asarray(Wv_b, np.float32)
             + np.asarray(Wo_b, np.float32)).astype(np.float32)
    out = np.empty((B, S, D), np.float32)
    for b in range(B):
        acc = res.results[4 * b]["pout"].astype(np.float32)
        for g in range(1, 4):
            acc = acc + res.results[4 * b + g]["pout"]
        out[b] = acc + obias
    return out


# revision 12
# speedup vs baseline: 1.8521x; 1.0624x over previous
"""Multi-head self-attention on 8 Trainium2 NeuronCores.

Problem: B=2, S=2048, D=1024, H=16 heads (DK=64), fp32.

Sharding (8 cores): core c handles batch b = c//4 and head group g = c%4
(4 heads = 256 of the 1024 projection dims).  QKV are column-parallel,
Wo is row-parallel; the 4 partial outputs per batch are summed on the
host (cheap numpy add) together with a folded constant bias vector.

Device kernel (per core, identical SPMD program), bf16 matmul operands:
  - inputs are pre-transposed and pre-cast to bf16 on host (no on-device
    transposes): xT [1024,2048], wqT/wkT/wvT [1024,256], woT [256,1024].
  - V is projected for all 4 local heads up front; Q^T/K^T are projected
    PER HEAD, software-pipelined as TensorE filler inside the previous
    head's attention loop.  This keeps the PE continuously busy while
    ScalarE runs the exps, so the HAM clock gate stays at 2.4 GHz
    (an ACT-bound attention loop lets the PE micro-idle, HAM rethrottles
    to 1.2 GHz, and every matmul doubles in cost — measured 485us).
  - scores^T layout [kk, q] per (head, q-half): matmul -> PSUM[128,1024],
    exp(s/8 + mask_bias) fused on ScalarE -> bf16 P^T tiles,
    P^T @ V' (ones-column appended to V) accumulates context^T and the
    softmax denominators in one PSUM tile.
  - context is evicted unnormalized; the denominator row is reshaped to
    [128, 16] via a DRAM bounce so the iterative-divide reciprocal runs
    on 128 lanes (a [1, 2048] reciprocal costs ~13us), then broadcast
    back along partitions by DMA and applied with one tensor multiply.
  - Wo projection of normalized context^T -> partial out [2048, 1024].

PSUM budget (8 banks): score tiles [128,1024] x2 bufs (4) + context
accumulator [128,1024] (2) + projection tiles [128,512] x2 bufs (2).

Math notes (exactness):
  - K bias cancels in softmax (adds a per-query constant to scores).
  - V bias commutes: softmax(S) @ (V + 1 b_v^T) = softmax(S) @ V + b_v^T,
    so it is added on the host as Wo_w @ Wv_b (+ Wo_b) once per batch.
"""

import sys

for _p in ("/root/.axon_site", "/root/.axon_site/_ro/trn_rl_repo",
           "/root/.axon_site/_ro/pypackages", "/opt/trn_rl_repo"):
    if _p not in sys.path:
        sys.path.append(_p)

import ml_dtypes
import numpy as np

import concourse.bass as bass
import concourse.tile as tile
from concourse import bacc, mybir
from concourse.bass_utils import run_bass_kernel_spmd

B, S, D, H = 2, 2048, 1024, 16
DK = D // H          # 64 head dim
NCORES = 8
HL = H // 4          # 4 heads per core
CL = HL * DK         # 256 local context dims per core
P = 128
EC = D // P          # 8 contraction chunks
F32 = mybir.dt.float32
BF16 = mybir.dt.bfloat16
AF = mybir.ActivationFunctionType
BF = ml_dtypes.bfloat16

KT_TILES = S // P    # 16 key tiles
QW = 512             # matmul moving-dim chunk
SCW = 1024           # score-tile q width (one PSUM score tile)
NQH = S // SCW       # 2 q-halves per head

LAST_RESULT = None   # BassKernelResults of the most recent run (for test.py)


def build_program():
    nc = bacc.Bacc("TRN2", target_bir_lowering=False, debug=False,
                   num_devices=NCORES)
    xT = nc.dram_tensor("xT", [D, S], BF16, kind="ExternalInput")
    wqT = nc.dram_tensor("wqT", [D, CL], BF16, kind="ExternalInput")
    wkT = nc.dram_tensor("wkT", [D, CL], BF16, kind="ExternalInput")
    wvT = nc.dram_tensor("wvT", [D, CL], BF16, kind="ExternalInput")
    bq4 = nc.dram_tensor("bq4", [DK, HL], F32, kind="ExternalInput")
    mb = nc.dram_tensor("mb", [P, KT_TILES], F32, kind="ExternalInput")
    woT = nc.dram_tensor("woT", [CL, D], BF16, kind="ExternalInput")
    pout = nc.dram_tensor("pout", [S, D], F32, kind="ExternalOutput")

    with tile.TileContext(nc) as tc:
        with (
            tc.tile_pool(name="consts", bufs=1) as consts,
            tc.tile_pool(name="work", bufs=1) as work,
            tc.tile_pool(name="psum", bufs=1, space="PSUM") as psum,
            tc.tile_pool(name="dramp", bufs=2, space="DRAM") as dramp,
        ):
            # persistent SBUF tensors
            xt_sb = consts.tile([P, EC, S], BF16)
            wq_sb = consts.tile([P, EC, CL], BF16)
            wk_sb = consts.tile([P, EC, CL], BF16)
            wv_sb = consts.tile([P, EC, CL], BF16)
            v_sb = consts.tile([P, KT_TILES, HL, DK + 1], BF16)  # V + ones col
            ctxn = consts.tile([P, 2, S], BF16)                  # normalized ctx^T
            bq_sb = consts.tile([DK, HL], F32)
            mb_sb = consts.tile([P, KT_TILES], F32)
            wo_sb = consts.tile([P, 2, D], BF16)

            # chunked load: V-projection matmuls can start after chunk 0
            for e in range(EC):
                nc.sync.dma_start(
                    out=xt_sb[:, e, :],
                    in_=xT.rearrange("(j p) q -> p j q", p=P)[:, e, :])
            nc.sync.dma_start(out=wq_sb, in_=wqT.rearrange("(j p) c -> p j c", p=P))
            nc.sync.dma_start(out=wk_sb, in_=wkT.rearrange("(j p) c -> p j c", p=P))
            nc.sync.dma_start(out=wv_sb, in_=wvT.rearrange("(j p) c -> p j c", p=P))
            nc.sync.dma_start(out=bq_sb, in_=bq4[:, :])
            nc.sync.dma_start(out=mb_sb, in_=mb[:, :])
            nc.sync.dma_start(out=wo_sb, in_=woT.rearrange("(j p) c -> p j c", p=P))
            nc.vector.memset(v_sb[:, :, :, DK:DK + 1], 1.0)

            # ---- V projection, all local heads up front ----
            for kt in range(KT_TILES):
                ps = psum.tile([P, QW], F32, tag="pj", bufs=2, name=f"pv{kt}")
                for e in range(EC):
                    nc.tensor.matmul(
                        ps[:, 0:CL],
                        lhsT=xt_sb[:, e, kt * P:(kt + 1) * P],
                        rhs=wv_sb[:, e, :],
                        start=(e == 0), stop=(e == EC - 1))
                nc.vector.tensor_copy(
                    out=v_sb[:, kt, :, 0:DK],
                    in_=ps[:, 0:CL].rearrange("p (h d) -> p h d", h=HL))

            # Per-head Q^T/K^T projection emitters.  Each returns a list of
            # closures (one matmul group each) so the caller can interleave
            # them as TensorE filler inside the previous head's attention.
            def qk_groups(h, qt_t, kt_t):
                groups = []
                for w_sb, o_t, is_q in ((wq_sb, qt_t, True), (wk_sb, kt_t, False)):
                    for qc in range(S // QW):
                        def g(w_sb=w_sb, o_t=o_t, is_q=is_q, qc=qc, h=h):
                            ps = psum.tile([P, QW], F32, tag="pj", bufs=2,
                                           name=f"pqk{h}_{int(is_q)}_{qc}")
                            for e in range(EC):
                                nc.tensor.matmul(
                                    ps[0:DK, :],
                                    lhsT=w_sb[:, e, h * DK:(h + 1) * DK],
                                    rhs=xt_sb[:, e, qc * QW:(qc + 1) * QW],
                                    start=(e == 0), stop=(e == EC - 1))
                            dst = o_t[:, qc * QW:(qc + 1) * QW]
                            if is_q:
                                nc.vector.tensor_scalar_add(
                                    out=dst, in0=ps[0:DK, :],
                                    scalar1=bq_sb[:, h:h + 1])
                            else:
                                nc.vector.tensor_copy(out=dst, in_=ps[0:DK, :])
                        groups.append(g)
                return groups

            def alloc_qk(h):
                qt_t = work.tile([DK, S], BF16, tag="qt", bufs=2, name=f"qt{h}")
                kt_t = work.tile([DK, S], BF16, tag="kt", bufs=2, name=f"kt{h}")
                return qt_t, kt_t

            # head 0's projections run up front
            cur_qk = alloc_qk(0)
            for g in qk_groups(0, *cur_qk):
                g()

            scale = 1.0 / float(np.sqrt(DK))
            for h in range(HL):
                qt_t, kt_t = cur_qk
                if h + 1 < HL:
                    nxt_qk = alloc_qk(h + 1)
                    filler = qk_groups(h + 1, *nxt_qk)
                else:
                    nxt_qk, filler = None, []
                fi = 0

                hb, hr = h // 2, (h % 2) * DK
                it = 0
                for qh in range(NQH):
                    q0 = qh * SCW
                    ctx_ps = psum.tile([P, SCW], F32, tag="ctx", bufs=1,
                                       name=f"ctx{h}_{qh}")
                    for kt in range(KT_TILES):
                        sc_ps = psum.tile([P, SCW], F32, tag="sc", bufs=2,
                                          name=f"sc{h}_{qh}_{kt}")
                        for c in range(SCW // QW):
                            nc.tensor.matmul(
                                sc_ps[:, c * QW:(c + 1) * QW],
                                lhsT=kt_t[:, kt * P:(kt + 1) * P],
                                rhs=qt_t[:, q0 + c * QW:q0 + (c + 1) * QW],
                                start=True, stop=True)
                        pt = work.tile([P, SCW], BF16, tag="pt", bufs=3,
                                       name=f"pt{h}_{qh}_{kt}")
                        nc.scalar.activation(out=pt, in_=sc_ps, func=AF.Exp,
                                             bias=mb_sb[:, kt:kt + 1],
                                             scale=scale)
                        for c in range(SCW // QW):
                            nc.tensor.matmul(
                                ctx_ps[0:DK + 1, c * QW:(c + 1) * QW],
                                lhsT=v_sb[:, kt, h, :],
                                rhs=pt[:, c * QW:(c + 1) * QW],
                                start=(kt == 0), stop=(kt == KT_TILES - 1))
                        # TensorE filler: next head's Q/K projection groups
                        if it % 4 == 1 and fi < len(filler):
                            filler[fi]()
                            fi += 1
                        it += 1
                    # evict unnormalized context + denominator row, then
                    # normalize this q-half (the chain overlaps the next
                    # q-half / head attention; DMA bounce reshapes the
                    # denominator so the reciprocal runs on 128 lanes)
                    ctxu = work.tile([DK, SCW], F32, tag="ctxu", bufs=3,
                                     name=f"ctxu{h}_{qh}")
                    nc.vector.tensor_copy(out=ctxu, in_=ctx_ps[0:DK, :])
                    den = work.tile([1, SCW], F32, tag="den", bufs=3,
                                    name=f"den{h}_{qh}")
                    nc.vector.tensor_copy(out=den, in_=ctx_ps[DK:DK + 1, :])
                    dd = dramp.tile([1, SCW], F32, tag="dd", name=f"dd{h}{qh}")
                    nc.sync.dma_start(out=dd, in_=den)
                    den2 = work.tile([P, SCW // P], F32, tag="den2", bufs=3,
                                     name=f"den2{h}_{qh}")
                    nc.sync.dma_start(
                        out=den2, in_=dd.rearrange("o (p f) -> (o p) f", p=P))
                    den2r = work.tile([P, SCW // P], F32, tag="den2r", bufs=3,
                                      name=f"den2r{h}_{qh}")
                    nc.vector.reciprocal(out=den2r, in_=den2)
                    dr = dramp.tile([1, SCW], F32, tag="dr", name=f"dr{h}{qh}")
                    nc.sync.dma_start(
                        out=dr.rearrange("o (p f) -> (o p) f", p=P), in_=den2r)
                    rb = work.tile([DK, SCW], F32, tag="rb", bufs=3,
                                   name=f"rb{h}_{qh}")
                    nc.sync.dma_start(out=rb, in_=dr.to_broadcast([DK, SCW]))
                    nc.vector.tensor_mul(out=ctxn[hr:hr + DK, hb, q0:q0 + SCW],
                                         in0=ctxu, in1=rb)
                while fi < len(filler):
                    filler[fi]()
                    fi += 1
                cur_qk = nxt_qk

            # ---- output projection ----
            for t in range(S // P):
                po = work.tile([P, D], F32, tag="po", bufs=3, name=f"po{t}")
                for dc in range(2):
                    ps = psum.tile([P, QW], F32, tag="pj", bufs=2,
                                   name=f"pw{t}_{dc}")
                    for cb in range(2):
                        nc.tensor.matmul(
                            ps,
                            lhsT=ctxn[:, cb, t * P:(t + 1) * P],
                            rhs=wo_sb[:, cb, dc * QW:(dc + 1) * QW],
                            start=(cb == 0), stop=(cb == 1))
                    nc.vector.tensor_copy(out=po[:, dc * QW:(dc + 1) * QW], in_=ps)
                nc.sync.dma_start(out=pout[t * P:(t + 1) * P, :], in_=po)

    nc.compile()
    return nc


_PROGRAM = None


def _get_program():
    global _PROGRAM
    if _PROGRAM is None:
        _PROGRAM = build_program()
    return _PROGRAM


def _bf(a):
    return np.ascontiguousarray(np.asarray(a, np.float32)).astype(BF)


def kernel(x, mask, Wq_w, Wq_b, Wk_w, Wk_b, Wv_w, Wv_b, Wo_w, Wo_b,
           **run_kwargs):
    global LAST_RESULT
    x = np.asarray(x, np.float32)
    mask = np.asarray(mask)
    Wq_w = np.asarray(Wq_w, np.float32)
    Wk_w = np.asarray(Wk_w, np.float32)
    Wv_w = np.asarray(Wv_w, np.float32)
    Wo_w = np.asarray(Wo_w, np.float32)

    nc = _get_program()

    xTs = [_bf(x[b].T) for b in range(B)]
    mbs = []
    for b in range(B):
        mrow = np.asarray(mask[b, 0, 0, :])
        bias = np.where(mrow == 0, np.float32(-50.0), np.float32(0.0))
        mbs.append(np.ascontiguousarray(bias.reshape(S // P, P).T.astype(np.float32)))

    in_maps = []
    for c in range(NCORES):
        b, g = c // 4, c % 4
        sl = slice(g * CL, (g + 1) * CL)
        in_maps.append({
            "xT": xTs[b],
            "wqT": _bf(Wq_w[sl, :].T),
            "wkT": _bf(Wk_w[sl, :].T),
            "wvT": _bf(Wv_w[sl, :].T),
            "bq4": np.ascontiguousarray(
                np.asarray(Wq_b, np.float32)[sl].reshape(HL, DK).T),
            "mb": mbs[b],
            "woT": _bf(Wo_w[:, sl].T),
        })

    res = run_bass_kernel_spmd(nc, in_maps, core_ids=list(range(NCORES)),
                               **run_kwargs)
    LAST_RESULT = res

    # host-side unshard: sum the 4 row-parallel partials per batch and add
    # the folded constant bias (Wo @ Wv_b + Wo_b).
    obias = (Wo_w @ np.asarray(Wv_b, np.float32)
             + np.asarray(Wo_b, np.float32)).astype(np.float32)
    out = np.empty((B, S, D), np.float32)
    for b in range(B):
        acc = res.results[4 * b]["pout"].astype(np.float32)
        for g in range(1, 4):
            acc = acc + res.results[4 * b + g]["pout"]
        out[b] = acc + obias
    return out


# revision 13
# speedup vs baseline: 1.8530x; 1.0005x over previous
"""Multi-head self-attention on 8 Trainium2 NeuronCores.

Problem: B=2, S=2048, D=1024, H=16 heads (DK=64), fp32.

Sharding (8 cores): core c handles batch b = c//4 and head group g = c%4
(4 heads = 256 of the 1024 projection dims).  QKV are column-parallel,
Wo is row-parallel; the 4 partial outputs per batch are summed on the
host (cheap numpy add) together with a folded constant bias vector.

Device kernel (per core, identical SPMD program), bf16 matmul operands:
  - inputs are pre-transposed and pre-cast to bf16 on host (no on-device
    transposes): xT [1024,2048], wqT/wkT/wvT [1024,256], woT [256,1024].
  - V is projected for all 4 local heads up front; Q^T/K^T are projected
    PER HEAD, software-pipelined as TensorE filler inside the previous
    head's attention loop.  This keeps the PE continuously busy while
    ScalarE runs the exps, so the HAM clock gate stays at 2.4 GHz
    (an ACT-bound attention loop lets the PE micro-idle, HAM rethrottles
    to 1.2 GHz, and every matmul doubles in cost — measured 485us).
  - scores^T layout [kk, q] per (head, q-half): matmul -> PSUM[128,1024],
    exp(s/8 + mask_bias) fused on ScalarE -> bf16 P^T tiles,
    P^T @ V' (ones-column appended to V) accumulates context^T and the
    softmax denominators in one PSUM tile.
  - context is evicted unnormalized; the denominator row is reshaped to
    [128, 16] via a DRAM bounce so the iterative-divide reciprocal runs
    on 128 lanes (a [1, 2048] reciprocal costs ~13us), then broadcast
    back along partitions by DMA and applied with one tensor multiply.
  - Wo projection of normalized context^T -> partial out [2048, 1024].

PSUM budget (8 banks): score tiles [128,1024] x2 bufs (4) + context
accumulator [128,1024] (2) + projection tiles [128,512] x2 bufs (2).

Math notes (exactness):
  - K bias cancels in softmax (adds a per-query constant to scores).
  - V bias commutes: softmax(S) @ (V + 1 b_v^T) = softmax(S) @ V + b_v^T,
    so it is added on the host as Wo_w @ Wv_b (+ Wo_b) once per batch.
"""

import sys

for _p in ("/root/.axon_site", "/root/.axon_site/_ro/trn_rl_repo",
           "/root/.axon_site/_ro/pypackages", "/opt/trn_rl_repo"):
    if _p not in sys.path:
        sys.path.append(_p)

import ml_dtypes
import numpy as np

import concourse.bass as bass
import concourse.tile as tile
from concourse import bacc, mybir
from concourse.bass_utils import run_bass_kernel_spmd

B, S, D, H = 2, 2048, 1024, 16
DK = D // H          # 64 head dim
NCORES = 8
HL = H // 4          # 4 heads per core
CL = HL * DK         # 256 local context dims per core
P = 128
EC = D // P          # 8 contraction chunks
F32 = mybir.dt.float32
BF16 = mybir.dt.bfloat16
AF = mybir.ActivationFunctionType
BF = ml_dtypes.bfloat16

KT_TILES = S // P    # 16 key tiles
QW = 512             # matmul moving-dim chunk
SCW = 1024           # score-tile q width (one PSUM score tile)
NQH = S // SCW       # 2 q-halves per head

LAST_RESULT = None   # BassKernelResults of the most recent run (for test.py)


def build_program():
    nc = bacc.Bacc("TRN2", target_bir_lowering=False, debug=False,
                   num_devices=NCORES)
    xT = nc.dram_tensor("xT", [D, S], BF16, kind="ExternalInput")
    wqT = nc.dram_tensor("wqT", [D, CL], BF16, kind="ExternalInput")
    wkT = nc.dram_tensor("wkT", [D, CL], BF16, kind="ExternalInput")
    wvT = nc.dram_tensor("wvT", [D, CL], BF16, kind="ExternalInput")
    bq4 = nc.dram_tensor("bq4", [DK, HL], F32, kind="ExternalInput")
    mb = nc.dram_tensor("mb", [P, KT_TILES], F32, kind="ExternalInput")
    woT = nc.dram_tensor("woT", [CL, D], BF16, kind="ExternalInput")
    pout = nc.dram_tensor("pout", [S, D], F32, kind="ExternalOutput")

    with tile.TileContext(nc) as tc:
        with (
            tc.tile_pool(name="consts", bufs=1) as consts,
            tc.tile_pool(name="work", bufs=1) as work,
            tc.tile_pool(name="psum", bufs=1, space="PSUM") as psum,
            tc.tile_pool(name="dramp", bufs=2, space="DRAM") as dramp,
        ):
            # persistent SBUF tensors
            xt_sb = consts.tile([P, EC, S], BF16)
            wq_sb = consts.tile([P, EC, CL], BF16)
            wk_sb = consts.tile([P, EC, CL], BF16)
            wv_sb = consts.tile([P, EC, CL], BF16)
            v_sb = consts.tile([P, KT_TILES, HL, DK + 1], BF16)  # V + ones col
            ctxn = consts.tile([P, 2, S], BF16)                  # normalized ctx^T
            bq_sb = consts.tile([DK, HL], F32)
            mb_sb = consts.tile([P, KT_TILES], F32)
            wo_sb = consts.tile([P, 2, D], BF16)

            # load order matters: the V projection (first PE work) needs wv
            # and xt chunk e as its e-loop reaches it, so those go first.
            nc.sync.dma_start(out=wv_sb, in_=wvT.rearrange("(j p) c -> p j c", p=P))
            xr = xT.rearrange("(j p) q -> p j q", p=P)
            nc.sync.dma_start(out=xt_sb[:, 0, :], in_=xr[:, 0, :])
            nc.sync.dma_start(out=xt_sb[:, 1, :], in_=xr[:, 1, :])
            nc.sync.dma_start(out=wq_sb, in_=wqT.rearrange("(j p) c -> p j c", p=P))
            for e in range(2, EC):
                nc.sync.dma_start(out=xt_sb[:, e, :], in_=xr[:, e, :])
            nc.sync.dma_start(out=wk_sb, in_=wkT.rearrange("(j p) c -> p j c", p=P))
            nc.sync.dma_start(out=bq_sb, in_=bq4[:, :])
            nc.sync.dma_start(out=mb_sb, in_=mb[:, :])
            nc.sync.dma_start(out=wo_sb, in_=woT.rearrange("(j p) c -> p j c", p=P))
            nc.vector.memset(v_sb[:, :, :, DK:DK + 1], 1.0)

            # ---- V projection, all local heads up front ----
            for kt in range(KT_TILES):
                ps = psum.tile([P, QW], F32, tag="pj", bufs=2, name=f"pv{kt}")
                for e in range(EC):
                    nc.tensor.matmul(
                        ps[:, 0:CL],
                        lhsT=xt_sb[:, e, kt * P:(kt + 1) * P],
                        rhs=wv_sb[:, e, :],
                        start=(e == 0), stop=(e == EC - 1))
                nc.vector.tensor_copy(
                    out=v_sb[:, kt, :, 0:DK],
                    in_=ps[:, 0:CL].rearrange("p (h d) -> p h d", h=HL))

            # Per-head Q^T/K^T projection emitters.  Each returns a list of
            # closures (one matmul group each) so the caller can interleave
            # them as TensorE filler inside the previous head's attention.
            def qk_groups(h, qt_t, kt_t):
                groups = []
                for w_sb, o_t, is_q in ((wq_sb, qt_t, True), (wk_sb, kt_t, False)):
                    for qc in range(S // QW):
                        def g(w_sb=w_sb, o_t=o_t, is_q=is_q, qc=qc, h=h):
                            ps = psum.tile([P, QW], F32, tag="pj", bufs=2,
                                           name=f"pqk{h}_{int(is_q)}_{qc}")
                            for e in range(EC):
                                nc.tensor.matmul(
                                    ps[0:DK, :],
                                    lhsT=w_sb[:, e, h * DK:(h + 1) * DK],
                                    rhs=xt_sb[:, e, qc * QW:(qc + 1) * QW],
                                    start=(e == 0), stop=(e == EC - 1))
                            dst = o_t[:, qc * QW:(qc + 1) * QW]
                            if is_q:
                                nc.vector.tensor_scalar_add(
                                    out=dst, in0=ps[0:DK, :],
                                    scalar1=bq_sb[:, h:h + 1])
                            else:
                                nc.vector.tensor_copy(out=dst, in_=ps[0:DK, :])
                        groups.append(g)
                return groups

            def alloc_qk(h):
                qt_t = work.tile([DK, S], BF16, tag="qt", bufs=2, name=f"qt{h}")
                kt_t = work.tile([DK, S], BF16, tag="kt", bufs=2, name=f"kt{h}")
                return qt_t, kt_t

            # head 0's projections run up front
            cur_qk = alloc_qk(0)
            for g in qk_groups(0, *cur_qk):
                g()

            scale = 1.0 / float(np.sqrt(DK))
            for h in range(HL):
                qt_t, kt_t = cur_qk
                if h + 1 < HL:
                    nxt_qk = alloc_qk(h + 1)
                    filler = qk_groups(h + 1, *nxt_qk)
                else:
                    nxt_qk, filler = None, []
                fi = 0

                hb, hr = h // 2, (h % 2) * DK
                it = 0
                for qh in range(NQH):
                    q0 = qh * SCW
                    ctx_ps = psum.tile([P, SCW], F32, tag="ctx", bufs=1,
                                       name=f"ctx{h}_{qh}")
                    for kt in range(KT_TILES):
                        sc_ps = psum.tile([P, SCW], F32, tag="sc", bufs=2,
                                          name=f"sc{h}_{qh}_{kt}")
                        for c in range(SCW // QW):
                            nc.tensor.matmul(
                                sc_ps[:, c * QW:(c + 1) * QW],
                                lhsT=kt_t[:, kt * P:(kt + 1) * P],
                                rhs=qt_t[:, q0 + c * QW:q0 + (c + 1) * QW],
                                start=True, stop=True)
                        pt = work.tile([P, SCW], BF16, tag="pt", bufs=3,
                                       name=f"pt{h}_{qh}_{kt}")
                        nc.scalar.activation(out=pt, in_=sc_ps, func=AF.Exp,
                                             bias=mb_sb[:, kt:kt + 1],
                                             scale=scale)
                        for c in range(SCW // QW):
                            nc.tensor.matmul(
                                ctx_ps[0:DK + 1, c * QW:(c + 1) * QW],
                                lhsT=v_sb[:, kt, h, :],
                                rhs=pt[:, c * QW:(c + 1) * QW],
                                start=(kt == 0), stop=(kt == KT_TILES - 1))
                        # TensorE filler: next head's Q/K projection groups
                        if it % 4 == 1 and fi < len(filler):
                            filler[fi]()
                            fi += 1
                        it += 1
                    # evict unnormalized context + denominator row, then
                    # normalize this q-half (the chain overlaps the next
                    # q-half / head attention; DMA bounce reshapes the
                    # denominator so the reciprocal runs on 128 lanes)
                    ctxu = work.tile([DK, SCW], F32, tag="ctxu", bufs=3,
                                     name=f"ctxu{h}_{qh}")
                    nc.vector.tensor_copy(out=ctxu, in_=ctx_ps[0:DK, :])
                    den = work.tile([1, SCW], F32, tag="den", bufs=3,
                                    name=f"den{h}_{qh}")
                    nc.vector.tensor_copy(out=den, in_=ctx_ps[DK:DK + 1, :])
                    dd = dramp.tile([1, SCW], F32, tag="dd", name=f"dd{h}{qh}")
                    nc.sync.dma_start(out=dd, in_=den)
                    den2 = work.tile([P, SCW // P], F32, tag="den2", bufs=3,
                                     name=f"den2{h}_{qh}")
                    nc.sync.dma_start(
                        out=den2, in_=dd.rearrange("o (p f) -> (o p) f", p=P))
                    den2r = work.tile([P, SCW // P], F32, tag="den2r", bufs=3,
                                      name=f"den2r{h}_{qh}")
                    nc.vector.reciprocal(out=den2r, in_=den2)
                    dr = dramp.tile([1, SCW], F32, tag="dr", name=f"dr{h}{qh}")
                    nc.sync.dma_start(
                        out=dr.rearrange("o (p f) -> (o p) f", p=P), in_=den2r)
                    rb = work.tile([DK, SCW], F32, tag="rb", bufs=3,
                                   name=f"rb{h}_{qh}")
                    nc.sync.dma_start(out=rb, in_=dr.to_broadcast([DK, SCW]))
                    nc.vector.tensor_mul(out=ctxn[hr:hr + DK, hb, q0:q0 + SCW],
                                         in0=ctxu, in1=rb)
                while fi < len(filler):
                    filler[fi]()
                    fi += 1
                cur_qk = nxt_qk

            # ---- output projection ----
            for t in range(S // P):
                po = work.tile([P, D], F32, tag="po", bufs=3, name=f"po{t}")
                for dc in range(2):
                    ps = psum.tile([P, QW], F32, tag="pj", bufs=2,
                                   name=f"pw{t}_{dc}")
                    for cb in range(2):
                        nc.tensor.matmul(
                            ps,
                            lhsT=ctxn[:, cb, t * P:(t + 1) * P],
                            rhs=wo_sb[:, cb, dc * QW:(dc + 1) * QW],
                            start=(cb == 0), stop=(cb == 1))
                    nc.vector.tensor_copy(out=po[:, dc * QW:(dc + 1) * QW], in_=ps)
                nc.sync.dma_start(out=pout[t * P:(t + 1) * P, :], in_=po)

    nc.compile()
    return nc


_PROGRAM = None


def _get_program():
    global _PROGRAM
    if _PROGRAM is None:
        _PROGRAM = build_program()
    return _PROGRAM


def _bf(a):
    return np.ascontiguousarray(np.asarray(a, np.float32)).astype(BF)


def kernel(x, mask, Wq_w, Wq_b, Wk_w, Wk_b, Wv_w, Wv_b, Wo_w, Wo_b,
           **run_kwargs):
    global LAST_RESULT
    x = np.asarray(x, np.float32)
    mask = np.asarray(mask)
    Wq_w = np.asarray(Wq_w, np.float32)
    Wk_w = np.asarray(Wk_w, np.float32)
    Wv_w = np.asarray(Wv_w, np.float32)
    Wo_w = np.asarray(Wo_w, np.float32)

    nc = _get_program()

    xTs = [_bf(x[b].T) for b in range(B)]
    mbs = []
    for b in range(B):
        mrow = np.asarray(mask[b, 0, 0, :])
        bias = np.where(mrow == 0, np.float32(-50.0), np.float32(0.0))
        mbs.append(np.ascontiguousarray(bias.reshape(S // P, P).T.astype(np.float32)))

    in_maps = []
    for c in range(NCORES):
        b, g = c // 4, c % 4
        sl = slice(g * CL, (g + 1) * CL)
        in_maps.append({
            "xT": xTs[b],
            "wqT": _bf(Wq_w[sl, :].T),
            "wkT": _bf(Wk_w[sl, :].T),
            "wvT": _bf(Wv_w[sl, :].T),
            "bq4": np.ascontiguousarray(
                np.asarray(Wq_b, np.float32)[sl].reshape(HL, DK).T),
            "mb": mbs[b],
            "woT": _bf(Wo_w[:, sl].T),
        })

    res = run_bass_kernel_spmd(nc, in_maps, core_ids=list(range(NCORES)),
                               **run_kwargs)
    LAST_RESULT = res

    # host-side unshard: sum the 4 row-parallel partials per batch and add
    # the folded constant bias (Wo @ Wv_b + Wo_b).
    obias = (Wo_w @ np.asarray(Wv_b, np.float32)
             + np.asarray(Wo_b, np.float32)).astype(np.float32)
    out = np.empty((B, S, D), np.float32)
    for b in range(B):
        acc = res.results[4 * b]["pout"].astype(np.float32)
        for g in range(1, 4):
            acc = acc + res.results[4 * b + g]["pout"]
        out[b] = acc + obias
    return out
